# revision 1
# baseline (speedup 1.0000x reference)
"""Trainium2 Bass kernel for a 3-layer GCN+RGCN GNN (IGMC-style).

Contract: kernel(**inputs) takes FULL unsharded inputs (same keys as
setup_inputs()) and returns the FULL [100000, 64] float32 output.

Strategy (8 NeuronCores, SPMD):
  - Nodes sharded by destination: core k owns nodes [k*12500, (k+1)*12500),
    padded to 12544 = 98 tiles x 128 partitions.
  - Per message-passing round every core holds a replicated fp32 node table
    in local DRAM, rebuilt by a small AllGather of per-core shards.
  - Edge messages are fetched with the GPSIMD dma_gather custom op (one
    256B row per edge slot).  int16 indices limit a gather to 32K rows, so
    the table is split into 4 segments and edges are grouped by
    (dst-tile[, relation], src-segment), each group padded to 128-edge
    chunks (pad: index 0 + dst -1, neutralized by the indicator matmul).
  - Segment-sum runs on the tensor engine: per 128-edge chunk a bf16 0/1
    indicator S[e, m] = (dst_local[e] == m) is built on the vector engine
    and matmul-accumulated into PSUM.
  - GCN rounds gather a pre-transformed y = h @ Wn table (matmul commutes
    with segment-sum).  RGCN rounds gather raw h, accumulate per-relation
    transposed aggregates, then apply the 5 relation weights as small fp32
    matmuls.
  - Host-side preprocessing only does index manipulation / layout packing;
    all model FLOPs run on device.
"""

import sys
import math
import numpy as np

sys.path.insert(0, "/opt/trn_rl_repo")

import ml_dtypes

BF16 = ml_dtypes.bfloat16

N_CORES = 8
SEG_MAX = 32768          # int16 index reach per dma_gather


class Cfg:
    def __init__(self, n_nodes=100000, n_edges=1600000, n_rel=5,
                 d_in=128, d_h=64, d_out=64, group_g=6, group_r=4,
                 mm_dtype="fp16", table_fp16=False):
        self.n_nodes = n_nodes
        self.n_edges = n_edges
        self.R = n_rel
        self.d_in = d_in
        self.d_h = d_h
        self.d_out = d_out
        self.own = n_nodes // N_CORES
        self.T = math.ceil(self.own / 128)
        self.pad = self.T * 128
        self.rows = N_CORES * 128 * self.T          # table rows
        self.n_seg = math.ceil(self.rows / SEG_MAX)
        assert self.rows % self.n_seg == 0
        self.seg_rows = self.rows // self.n_seg
        self.group_g = group_g
        self.group_r = group_r
        self.mm_dtype = mm_dtype
        self.table_fp16 = table_fp16
        if table_fp16:
            assert mm_dtype == "fp16"
        self.mm_bf16 = mm_dtype != "fp32" and not table_fp16  # cast needed
        assert n_nodes % N_CORES == 0


# ---------------------------------------------------- host preprocessing ---

def _row_ids(v, cfg):
    """Original node id -> row id in the all-gathered table.

    AG input per core is DRAM [128, T*64] (partition-major tiles); the
    concatenated table viewed as [N_CORES*128*T, 64] has
    row = (core*128 + p)*T + t  for local node l = t*128 + p."""
    k = v // cfg.own
    l = v - k * cfg.own
    t, p = l // 128, l % 128
    return (k * 128 + p) * cfg.T + t


def pack_round(src, dst, rel, R, cfg, B):
    """Group edges by (dst tile, rel, src segment); build per-core int16
    gather indices, bf16 dst-local arrays, and the batch/chunk metadata
    shared by all cores (chunk counts are maxed over cores)."""
    NS = cfg.n_seg
    T = cfg.T
    core = dst // cfg.own
    l = dst - core * cfg.own
    tile = l // 128
    dloc = (l % 128).astype(np.float32)
    rows = _row_ids(src, cfg)
    seg = rows // cfg.seg_rows
    lrow = rows - seg * cfg.seg_rows            # int16-safe local row
    gr = rel if R > 1 else np.zeros_like(tile)

    # group id: (core, tile, rel, seg)
    gid = (((core * T + tile) * R + gr) * NS + seg).astype(np.int64)
    n_groups = N_CORES * T * R * NS
    order = np.argsort(gid, kind="stable")
    gid_s = gid[order]
    lrow_s = lrow[order]
    dloc_s = dloc[order]
    counts = np.bincount(gid_s, minlength=n_groups).reshape(N_CORES, T, R, NS)
    starts = np.concatenate([[0], np.cumsum(counts.ravel())])

    K = np.ceil(counts.max(axis=0) / 128).astype(np.int64)   # [T, R, NS]

    # ---- batch/column layout (shared across cores) ----
    batches = []
    idx_c = 0
    chunk_c = 0
    NB = math.ceil(T / B)
    for b in range(NB):
        ts = list(range(b * B, min((b + 1) * B, T)))
        segs = []
        tile_chunks = {t: [] for t in ts}
        gbase = {}                     # (t, r, s) -> global chunk col
        c0 = chunk_c
        i0 = idx_c
        for s in range(NS):
            nch = 0
            for t in ts:
                for r in range(R):
                    k = int(K[t, r, s])
                    if k == 0:
                        continue
                    gbase[(t, r, s)] = chunk_c
                    for j in range(k):
                        tile_chunks[t].append((chunk_c + j - c0, r))
                    chunk_c += k
                    nch += k
            if nch:
                segs.append(dict(s=s, n_chunks=nch, idx_c0=idx_c - i0,
                                 chunk_c0=(chunk_c - nch) - c0))
                idx_c += nch * 8
        batches.append(dict(tiles=ts, idx_c0=i0, idx_cols=idx_c - i0,
                            chunk_c0=c0, chunks=chunk_c - c0, segs=segs,
                            tile_chunks=tile_chunks, gbase=gbase))
    IC, CC = idx_c, chunk_c

    # ---- per-core slot filling ----
    idx16s, dstvs = [], []
    # instruction-relative idx position base per group
    for k in range(N_CORES):
        idxf = np.zeros((IC // 8) * 128, np.int16)     # flat idx stream
        dv = np.full((128, CC), -1.0, np.float32)
        for binfo in batches:
            iseg = {sd["s"]: sd for sd in binfo["segs"]}
            for (t, r, s), cb in binfo["gbase"].items():
                g = ((k * T + t) * R + r) * NS + s
                s0, e0 = starts[g], starts[g + 1]
                n = e0 - s0
                if n == 0:
                    continue
                pos = np.arange(n)
                p = pos % 128
                ch = cb + pos // 128
                dv[p, ch] = dloc_s[s0:e0]
                # flat idx stream: instruction at column C starts at C*16
                sd = iseg[s]
                base = (binfo["idx_c0"] + sd["idx_c0"]) * 16
                loc = (cb - (binfo["chunk_c0"] + sd["chunk_c0"])) * 128
                idxf[base + loc + pos] = lrow_s[s0:e0]
        # wrap: idx i -> [i%16, i//16], replicated over 8 partition groups
        idx16 = np.tile(idxf.reshape(-1, 16).T, (8, 1))
        idx16s.append(np.ascontiguousarray(idx16))
        dstvs.append(dv)
    meta = dict(batches=batches, IC=IC, CC=CC)
    return idx16s, dstvs, meta


def preprocess(inputs, cfg):
    x = np.asarray(inputs["x"], np.float32)
    ei = np.asarray(inputs["edge_index"], np.int64)
    rei = np.asarray(inputs["rel_edge_index"], np.int64)
    ret = np.asarray(inputs["rel_edge_type"], np.int64)

    g_idx, g_dst, g_meta = pack_round(ei[0], ei[1], None, 1, cfg, cfg.group_g)
    r_idx, r_dst, r_meta = pack_round(rei[0], rei[1], ret, cfg.R, cfg,
                                      cfg.group_r)

    xTs = []
    for k in range(N_CORES):
        xo = np.zeros((cfg.pad, cfg.d_in), np.float32)
        xo[:cfg.own] = x[k * cfg.own:(k + 1) * cfg.own]
        xTs.append(np.ascontiguousarray(xo.T))

    D = cfg.d_h
    cols = {}
    pieces = []
    c = 0

    def put(name, w):
        nonlocal c
        w = np.asarray(w, np.float32)
        pad = np.zeros((cfg.d_in, w.shape[1]), np.float32)
        pad[:w.shape[0]] = w
        cols[name] = (c, w.shape[1], w.shape[0])
        pieces.append(pad)
        c += w.shape[1]

    for ll in range(3):
        put(f"gWs{ll}", inputs[f"gWs{ll}"])
        put(f"gWn{ll}", inputs[f"gWn{ll}"])
        put(f"rWr{ll}", inputs[f"rWr{ll}"])
        for r in range(cfg.R):
            put(f"rW{ll}_{r}", np.asarray(inputs[f"rW{ll}"], np.float32)[r])
    put("Wout", inputs["Wout"])
    w_pack = np.concatenate(pieces, axis=1)

    bnames = ["gb0", "rb0", "gb1", "rb1", "gb2", "rb2", "bout"]
    bias_rows = [np.asarray(inputs[n], np.float32) for n in bnames]
    bias_pack = np.tile(np.concatenate(bias_rows)[None, :], (128, 1))
    bcols = {n: i * D for i, n in enumerate(bnames)}

    sdt = {"bf16": BF16, "fp16": np.float16, "fp32": np.float32}[cfg.mm_dtype]
    iota = np.tile(np.arange(128, dtype=np.float32)[None, :],
                   (128, 1)).astype(sdt)
    ident = np.eye(128, dtype=np.float32)

    in_maps = []
    for k in range(N_CORES):
        in_maps.append({
            "xT": xTs[k],
            "g_idx": g_idx[k], "g_dst": g_dst[k].astype(sdt),
            "r_idx": r_idx[k], "r_dst": r_dst[k].astype(sdt),
            "w_pack": w_pack, "bias_pack": bias_pack,
            "iota": iota, "ident": ident,
        })
    meta = dict(g=g_meta, r=r_meta, wcols=cols, bcols=bcols)
    return in_maps, meta


# ------------------------------------------------------------- bass build ---

def build(cfg, meta, debug=False, stage=9):
    import concourse.bass as bass
    import concourse.bacc as bacc
    import concourse.mybir as mybir
    import concourse.tile as tile

    dt = mybir.dt
    D = cfg.d_h
    T = cfg.T
    MDT = {"bf16": dt.bfloat16, "fp16": dt.float16,
           "fp32": dt.float32}[cfg.mm_dtype]
    TBL = dt.float16 if cfg.table_fp16 else dt.float32
    ROWW = 128 if cfg.table_fp16 else 64
    g_meta, r_meta = meta["g"], meta["r"]
    wcols, bcols = meta["wcols"], meta["bcols"]

    nc = bacc.Bacc(None, target_bir_lowering=False, num_devices=N_CORES)

    xT_in = nc.dram_tensor("xT", [cfg.d_in, T * 128], dt.float32,
                           kind="ExternalInput")
    g_idx_in = nc.dram_tensor("g_idx", [128, g_meta["IC"]], dt.int16,
                              kind="ExternalInput")
    g_dst_in = nc.dram_tensor("g_dst", [128, g_meta["CC"]], MDT,
                              kind="ExternalInput")
    r_idx_in = nc.dram_tensor("r_idx", [128, r_meta["IC"]], dt.int16,
                              kind="ExternalInput")
    r_dst_in = nc.dram_tensor("r_dst", [128, r_meta["CC"]], MDT,
                              kind="ExternalInput")
    n_wcols = sum(v[1] for v in wcols.values())
    w_in = nc.dram_tensor("w_pack", [cfg.d_in, n_wcols], dt.float32,
                          kind="ExternalInput")
    b_in = nc.dram_tensor("bias_pack", [128, 7 * D], dt.float32,
                          kind="ExternalInput")
    iota_in = nc.dram_tensor("iota", [128, 128], MDT,
                             kind="ExternalInput")
    ident_in = nc.dram_tensor("ident", [128, 128], dt.float32,
                              kind="ExternalInput")
    out_ext = nc.dram_tensor("out", [128, T * cfg.d_out], dt.float32,
                             kind="ExternalOutput")

    hTg = nc.dram_tensor("hTg", [D, T * 128], dt.float32)
    hTr = nc.dram_tensor("hTr", [D, T * 128], dt.float32)
    agin, agout = {}, {}
    for name in ["y0", "h0", "y1", "h1", "y2", "h2"]:
        agin[name] = nc.dram_tensor(f"agin_{name}", [128, T * ROWW], TBL)
        agout[name] = nc.dram_tensor(f"agout_{name}", [cfg.rows, ROWW],
                                     TBL, addr_space="Shared")
    rg = [list(range(N_CORES))]

    if debug:
        dbg_y0 = nc.dram_tensor("dbg_y0", [cfg.rows, ROWW], TBL,
                                kind="ExternalOutput")
        dbg_h0 = nc.dram_tensor("dbg_h0", [128, T * ROWW], TBL,
                                kind="ExternalOutput")

    import contextlib
    with tile.TileContext(nc) as tc, contextlib.ExitStack() as ctx:
        cpool = ctx.enter_context(tc.tile_pool(name="consts", bufs=1))
        lpool = ctx.enter_context(tc.tile_pool(name="loads", bufs=2))
        gpool = ctx.enter_context(tc.tile_pool(name="gath", bufs=2))
        spool = ctx.enter_context(tc.tile_pool(name="smat", bufs=6))
        hpool = ctx.enter_context(tc.tile_pool(name="work", bufs=3))
        stpool = ctx.enter_context(tc.tile_pool(name="stage", bufs=2))
        ppool = ctx.enter_context(tc.tile_pool(name="psum", bufs=2,
                                               space="PSUM"))

        w_sb = cpool.tile([cfg.d_in, n_wcols], dt.float32, tag="wsb")
        nc.sync.dma_start(out=w_sb[:], in_=w_in[:])
        b_sb = cpool.tile([128, 7 * D], dt.float32, tag="bsb")
        nc.sync.dma_start(out=b_sb[:], in_=b_in[:])
        iota_sb = cpool.tile([128, 128], MDT, tag="iosb")
        nc.sync.dma_start(out=iota_sb[:], in_=iota_in[:])
        id_sb = cpool.tile([128, 128], dt.float32, tag="idsb")
        nc.sync.dma_start(out=id_sb[:], in_=ident_in[:])

        def W(name):
            c0, n, kdim = wcols[name]
            return w_sb[:kdim, c0:c0 + n]

        def B(name):
            c0 = bcols[name]
            return b_sb[:, c0:c0 + D]

        def emit_gather(out_ap3, in_ap, idxs_ap, num_idxs):
            eng = nc.gpsimd
            _in = eng.lower_ap_dma(in_ap, for_custom_bir_dma=True)
            _idx = eng.lower_ap(idxs_ap)
            _out = eng.lower_ap(out_ap3)
            stride_bytes = ROWW * mybir.dt.size(in_ap.dtype)
            return eng.add_instruction(mybir.InstDMAGatherAnt(
                name=nc.get_next_instruction_name(),
                ins=[*_in, _idx, eng.lower_val_access(eng.to_reg(num_idxs))],
                outs=[_out],
                transpose=False,
                num_idxs=num_idxs,
                elem_size=D,
                stride_bytes_256=stride_bytes // 256,
                gen_mode=0,
                single_packet=False,
                queue_num=0,
                sbuf_tokens_per_rank=0,
                sbuf_free_dim_per_rank=0,
                sbuf_free_dim_pad_per_rank=0,
                sbuf_byte_offset=0))

        def do_gathers(binfo, idx_tile, table, tag):
            """Issue per-segment dma_gathers for one batch; returns the
            gathered tile [128, chunks*D] in MDT (cast if table is fp32)."""
            import os
            nch = binfo["chunks"]
            gath = gpool.tile([128, max(g_mcc, r_mcc) * D], TBL,
                              tag="g_x")
            if "noga" in os.environ.get("GDBG", ""):
                z = gpool.tile([128, max(g_mcc, r_mcc) * D], MDT, tag="gz_x")
                nc.gpsimd.memset(z[:], 0.0)
                return z
            for sd in binfo["segs"]:
                s, n = sd["s"], sd["n_chunks"]
                c0 = sd["chunk_c0"]
                emit_gather(
                    gath[:, c0 * D:(c0 + n) * D].rearrange(
                        "p (c d) -> p c d", d=D),
                    table[s * cfg.seg_rows:(s + 1) * cfg.seg_rows, 0:D],
                    idx_tile[:, sd["idx_c0"]:sd["idx_c0"] + n * 8],
                    n * 128)
            if cfg.mm_bf16:
                gbf = gpool.tile([128, max(g_mcc, r_mcc) * D], MDT,
                                 tag="gb_x")
                nc.vector.tensor_copy(out=gbf[:, :nch * D],
                                      in_=gath[:, :nch * D])
                return gbf
            return gath

        def make_S(dst_tile, col):
            S = spool.tile([128, 128], MDT, tag="S")
            nc.vector.tensor_tensor(
                out=S[:],
                in0=dst_tile[:, col:col + 1].to_broadcast([128, 128]),
                in1=iota_sb[:],
                op=mybir.AluOpType.is_equal)
            return S

        def finish_h(psum_o, bias_ap):
            h = hpool.tile([128, D], dt.float32, tag="h")
            nc.vector.tensor_tensor(out=h[:], in0=psum_o[:], in1=bias_ap,
                                    op=mybir.AluOpType.add)
            nc.vector.tensor_relu(out=h[:], in_=h[:])
            return h

        def transpose_h(h):
            pt = ppool.tile([D, 128], dt.float32, tag="pt", space="PSUM")
            nc.tensor.transpose(out=pt[:], in_=h[:], identity=id_sb[:])
            hT = hpool.tile([D, 128], dt.float32, tag="hT")
            nc.vector.tensor_copy(out=hT[:], in_=pt[:])
            return hT

        # ============ prologue: y0 = x @ gWn0 ============
        NB = len(g_meta["batches"])
        g_mic = max(b["idx_cols"] for b in g_meta["batches"])
        g_mcc = max(b["chunks"] for b in g_meta["batches"])
        r_mic = max(b["idx_cols"] for b in r_meta["batches"])
        r_mcc = max(b["chunks"] for b in r_meta["batches"])
        for binfo in g_meta["batches"]:
            ts = binfo["tiles"]
            t0, n = ts[0], len(ts)
            xt = lpool.tile([cfg.d_in, cfg.group_g * 128], dt.float32, tag="xt")
            nc.sync.dma_start(out=xt[:, :n * 128],
                              in_=xT_in[:, t0 * 128:(t0 + n) * 128])
            stg = stpool.tile([128, cfg.group_g * D], TBL, tag="stg_y0")
            for i in range(n):
                py = ppool.tile([128, D], dt.float32, tag="py", space="PSUM")
                nc.tensor.matmul(out=py[:], lhsT=xt[:, i * 128:(i + 1) * 128],
                                 rhs=W("gWn0"), start=True, stop=True)
                nc.vector.tensor_copy(out=stg[:, i * D:(i + 1) * D],
                                      in_=py[:])
            nc.sync.dma_start(
                out=agin["y0"][:].rearrange(
                    "p (t w) -> p t w", w=ROWW)[:, t0:t0 + n, 0:D],
                in_=stg[:, :n * D].rearrange("p (t d) -> p t d", d=D))

        nc.gpsimd.collective_compute(
            "AllGather", mybir.AluOpType.bypass, replica_groups=rg,
            ins=[agin["y0"][:]], outs=[agout["y0"][:]])
        if debug:
            nc.sync.dma_start(out=dbg_y0[:], in_=agout["y0"][:])

        # ============ rounds ============
        for ll in range(3 if stage >= 9 else (1 if stage >= 1 else 0)):
            # ---- GCN ----
            for binfo in g_meta["batches"]:
                ts = binfo["tiles"]
                t0, n = ts[0], len(ts)
                nch = binfo["chunks"]
                idx_t = lpool.tile([128, g_mic], dt.int16, tag="gidx")
                nc.sync.dma_start(
                    out=idx_t[:, :binfo["idx_cols"]],
                    in_=g_idx_in[:, binfo["idx_c0"]:
                                 binfo["idx_c0"] + binfo["idx_cols"]])
                dst_t = lpool.tile([128, g_mcc], MDT, tag="gdst")
                nc.sync.dma_start(
                    out=dst_t[:, :nch],
                    in_=g_dst_in[:, binfo["chunk_c0"]:
                                 binfo["chunk_c0"] + nch])
                if ll == 0:
                    sT = lpool.tile([cfg.d_in, cfg.group_g * 128], dt.float32,
                                    tag="sTg0")
                    nc.sync.dma_start(out=sT[:, :n * 128],
                                      in_=xT_in[:, t0 * 128:(t0 + n) * 128])
                    sdim = cfg.d_in
                else:
                    sT = lpool.tile([D, cfg.group_g * 128], dt.float32,
                                    tag="sTg")
                    nc.sync.dma_start(out=sT[:, :n * 128],
                                      in_=hTr[:, t0 * 128:(t0 + n) * 128])
                    sdim = D
                gath = do_gathers(binfo, idx_t, agout[f"y{ll}"], "g")
                stg = stpool.tile([128, cfg.group_g * D], TBL,
                                  tag="stg_h")
                stgT = stpool.tile([D, cfg.group_g * 128], dt.float32,
                                   tag="stgT")
                import os
                GDBG = os.environ.get("GDBG", "")
                for i, t in enumerate(ts):
                    po = ppool.tile([128, D], dt.float32, tag="po",
                                    space="PSUM")
                    chunks = binfo["tile_chunks"][t] if "noseg" not in GDBG else []
                    nc.tensor.matmul(out=po[:],
                                     lhsT=sT[:sdim, i * 128:(i + 1) * 128],
                                     rhs=W(f"gWs{ll}"), start=True,
                                     stop=(len(chunks) == 0),
                                     skip_group_check=True)
                    for j, (col, _r) in enumerate(chunks):
                        S = make_S(dst_t, col)
                        if "nos" in GDBG:
                            rhs_mm = gath[:, col * D:(col + 1) * D]
                            lhs_mm = S[:]
                        else:
                            rhs_mm = gath[:, col * D:(col + 1) * D]
                            lhs_mm = S[:]
                        nc.tensor.matmul(
                            out=po[:], lhsT=lhs_mm,
                            rhs=rhs_mm,
                            start=False, stop=(j == len(chunks) - 1),
                            skip_group_check=True)
                    h = finish_h(po, B(f"gb{ll}"))
                    nc.vector.tensor_copy(out=stg[:, i * D:(i + 1) * D],
                                          in_=h[:])
                    hT = transpose_h(h)
                    nc.vector.tensor_copy(out=stgT[:, i * 128:(i + 1) * 128],
                                          in_=hT[:])
                nc.sync.dma_start(
                    out=agin[f"h{ll}"][:].rearrange(
                        "p (t w) -> p t w", w=ROWW)[:, t0:t0 + n, 0:D],
                    in_=stg[:, :n * D].rearrange("p (t d) -> p t d", d=D))
                nc.sync.dma_start(out=hTg[:, t0 * 128:(t0 + n) * 128],
                                  in_=stgT[:, :n * 128])

            if debug and ll == 0:
                nc.sync.dma_start(out=dbg_h0[:], in_=agin["h0"][:])
            if stage < 2 and stage < 9:
                break
            nc.gpsimd.collective_compute(
                "AllGather", mybir.AluOpType.bypass, replica_groups=rg,
                ins=[agin[f"h{ll}"][:]], outs=[agout[f"h{ll}"][:]])
            if stage < 3 and stage < 9:
                break

            # ---- RGCN ----
            NBr = len(r_meta["batches"])
            for binfo in r_meta["batches"]:
                ts = binfo["tiles"]
                t0, n = ts[0], len(ts)
                nch = binfo["chunks"]
                idx_t = lpool.tile([128, r_mic], dt.int16, tag="ridx")
                nc.sync.dma_start(
                    out=idx_t[:, :binfo["idx_cols"]],
                    in_=r_idx_in[:, binfo["idx_c0"]:
                                 binfo["idx_c0"] + binfo["idx_cols"]])
                dst_t = lpool.tile([128, r_mcc], MDT, tag="rdst")
                nc.sync.dma_start(
                    out=dst_t[:, :nch],
                    in_=r_dst_in[:, binfo["chunk_c0"]:
                                 binfo["chunk_c0"] + nch])
                sT = lpool.tile([D, cfg.group_r * 128], dt.float32, tag="sTr")
                nc.sync.dma_start(out=sT[:, :n * 128],
                                  in_=hTg[:, t0 * 128:(t0 + n) * 128])
                gath = do_gathers(binfo, idx_t, agout[f"h{ll}"], "r")
                stgT = stpool.tile([D, cfg.group_r * 128], dt.float32,
                                   tag="stgTr")
                stg = stpool.tile([128, cfg.group_r * D], TBL, tag="stg_y2")
                stg_out = stpool.tile([128, cfg.group_r * D], dt.float32,
                                      tag="stg_out")
                for i, t in enumerate(ts):
                    po = ppool.tile([128, D], dt.float32, tag="po",
                                    space="PSUM")
                    nc.tensor.matmul(out=po[:],
                                     lhsT=sT[:, i * 128:(i + 1) * 128],
                                     rhs=W(f"rWr{ll}"), start=True,
                                     stop=False, skip_group_check=True)
                    by_rel = {}
                    for col, r in binfo["tile_chunks"][t]:
                        by_rel.setdefault(r, []).append(col)
                    rels = sorted(by_rel)
                    for ri, r in enumerate(rels):
                        cols = by_rel[r]
                        pa = ppool.tile([D, 128], dt.float32, tag="pa",
                                        space="PSUM")
                        for j, col in enumerate(cols):
                            S = make_S(dst_t, col)
                            nc.tensor.matmul(
                                out=pa[:],
                                lhsT=gath[:, col * D:(col + 1) * D],
                                rhs=S[:],
                                start=(j == 0), stop=(j == len(cols) - 1),
                                skip_group_check=True)
                        aggT = hpool.tile([D, 128], dt.float32, tag="aggT")
                        nc.vector.tensor_copy(out=aggT[:], in_=pa[:])
                        nc.tensor.matmul(out=po[:], lhsT=aggT[:],
                                         rhs=W(f"rW{ll}_{r}"), start=False,
                                         stop=(ri == len(rels) - 1),
                                         skip_group_check=True)
                    h = finish_h(po, B(f"rb{ll}"))
                    hT = transpose_h(h)
                    nc.vector.tensor_copy(out=stgT[:, i * 128:(i + 1) * 128],
                                          in_=hT[:])
                    py = ppool.tile([128, D], dt.float32, tag="py",
                                    space="PSUM")
                    if ll < 2:
                        nc.tensor.matmul(out=py[:], lhsT=hT[:],
                                         rhs=W(f"gWn{ll + 1}"),
                                         start=True, stop=True)
                        nc.vector.tensor_copy(out=stg[:, i * D:(i + 1) * D],
                                              in_=py[:])
                    else:
                        nc.tensor.matmul(out=py[:], lhsT=hT[:],
                                         rhs=W("Wout"), start=True, stop=True)
                        ot = hpool.tile([128, D], dt.float32, tag="ot")
                        nc.vector.tensor_tensor(out=ot[:], in0=py[:],
                                                in1=B("bout"),
                                                op=mybir.AluOpType.add)
                        nc.vector.tensor_relu(
                            out=stg_out[:, i * D:(i + 1) * D], in_=ot[:])
                nc.sync.dma_start(out=hTr[:, t0 * 128:(t0 + n) * 128],
                                  in_=stgT[:, :n * 128])
                if ll < 2:
                    nc.sync.dma_start(
                        out=agin[f"y{ll + 1}"][:].rearrange(
                            "p (t w) -> p t w", w=ROWW)[:, t0:t0 + n, 0:D],
                        in_=stg[:, :n * D].rearrange("p (t d) -> p t d", d=D))
                else:
                    nc.sync.dma_start(out=out_ext[:, t0 * D:(t0 + n) * D],
                                      in_=stg_out[:, :n * D])
            if ll < 2:
                nc.gpsimd.collective_compute(
                    "AllGather", mybir.AluOpType.bypass, replica_groups=rg,
                    ins=[agin[f"y{ll + 1}"][:]],
                    outs=[agout[f"y{ll + 1}"][:]])

    nc.compile()
    return nc


# ---------------------------------------------------------------- driver ---

_CACHE = {}


def _run(inputs, cfg, debug=False, stage=9, trace=False, trace_cores=None):
    from concourse.bass_utils import run_bass_kernel_spmd

    in_maps, meta = preprocess(inputs, cfg)
    key = (cfg.n_nodes, cfg.n_edges, cfg.group_g, cfg.group_r, cfg.mm_dtype, cfg.table_fp16, debug, stage,
           meta["g"]["IC"], meta["r"]["IC"])
    if key not in _CACHE:
        _CACHE[key] = build(cfg, meta, debug=debug, stage=stage)
    nc = _CACHE[key]
    kw = {}
    if trace:
        kw = dict(trace=True, trace_cores=trace_cores or [0])
    res = run_bass_kernel_spmd(nc, in_maps, core_ids=list(range(N_CORES)),
                               **kw)
    outs = []
    for k in range(N_CORES):
        o = np.asarray(res.results[k]["out"])
        o = o.reshape(128, cfg.T, cfg.d_out).transpose(1, 0, 2)
        outs.append(o.reshape(cfg.pad, cfg.d_out)[:cfg.own])
    full = np.concatenate(outs, axis=0)
    return full, res


def kernel(**inputs):
    cfg = Cfg()
    full, _ = _run(inputs, cfg)
    return full.astype(np.float32)

